# revision 13
# baseline (speedup 1.0000x reference)
"""Trainium2 Bass kernel for nn_AdaptiveDirectionGenerator.

MLP head (768 -> 512 -> 256) with LayerNorm+ReLU, two output heads
(K*D direction generator + softmax attention), per-row attention
weighting and per-row Gram-Schmidt orthonormalization of K=4 vectors.

Strategy: pure data parallel over 8 NeuronCores (batch 16384 -> 2048
rows/core), params replicated. Per core the batch is processed in 16
tiles of 128 rows (rows on SBUF partitions).

 - Matmuls on TensorE with transposed activations as the stationary
   operand (x is pre-transposed on host; h1/h2 transposed on-chip via PE
   transpose). Biases are added via K=1 ones-matmuls into PSUM.
 - LayerNorm stats via bn_stats/bn_aggr; the per-feature affine + ReLU
   is fused into the mandatory PSUM->SBUF copy on ScalarE after the
   transpose (feature dim = partitions there, so gamma/beta are
   per-partition scalars).
 - Softmax skips the max-subtraction (pre-activations are O(1)); exp
   with fused row-sum via activation(accum_out=...). Gram-Schmidt is
   invariant to the positive per-row softmax normalization, so the
   1/sum scale is only applied to the attn output.
 - Gram-Schmidt in "Cholesky form": the 10 Gram entries of the
   attention-weighted vectors come from fused product+row-sum pairs
   (tensor_tensor + tensor_scalar(accum_out), diagonals partially via
   ScalarE Square(accum_out)); the tiny per-partition Cholesky runs
   batched across a group of row-tiles; reconstruction uses fused
   scalar_tensor_tensor MACs.

Compute is bf16 (inputs converted on host), stats/coefficients in f32.
Outputs are written bf16 and upcast to f32 on the host.
"""

import sys

sys.path.insert(0, "/opt/trn_rl_repo")

import numpy as np
import ml_dtypes

import concourse.bass as bass
import concourse.mybir as mybir
import concourse.tile as tile
from concourse import bacc
from concourse.bass import ts
from concourse.bass_utils import run_bass_kernel_spmd
from concourse.masks import make_identity

BF16 = mybir.dt.bfloat16
F32 = mybir.dt.float32
NPBF16 = ml_dtypes.bfloat16

D, K, H1, H2 = 768, 4, 512, 256
KD = K * D
N_CORES = 8
B_FULL = 16384
P = 128

AL = mybir.AluOpType
AF = mybir.ActivationFunctionType


def build(nc, BL):
    """Emit the per-core program for a local batch of BL rows."""
    T = BL // P
    GRP = min(8, T)
    assert T % GRP == 0
    DC, H1C, H2C = D // P, H1 // P, H2 // P  # 6, 4, 2

    # ---------------- DRAM I/O ----------------
    xT = nc.dram_tensor("xT", [D, BL], BF16, kind="ExternalInput")
    w1 = nc.dram_tensor("w1", [D, H1], BF16, kind="ExternalInput")
    w2 = nc.dram_tensor("w2", [H1, H2], BF16, kind="ExternalInput")
    wd = nc.dram_tensor("wd", [H2, KD], BF16, kind="ExternalInput")
    wa = nc.dram_tensor("wa", [H2, D], BF16, kind="ExternalInput")
    b1r = nc.dram_tensor("b1r", [1, H1], BF16, kind="ExternalInput")
    b2r = nc.dram_tensor("b2r", [1, H2], BF16, kind="ExternalInput")
    bdr = nc.dram_tensor("bdr", [1, KD], BF16, kind="ExternalInput")
    bar = nc.dram_tensor("bar", [1, D], BF16, kind="ExternalInput")
    g1c = nc.dram_tensor("g1c", [P, H1C], F32, kind="ExternalInput")
    be1c = nc.dram_tensor("be1c", [P, H1C], F32, kind="ExternalInput")
    g2c = nc.dram_tensor("g2c", [P, H2C], F32, kind="ExternalInput")
    be2c = nc.dram_tensor("be2c", [P, H2C], F32, kind="ExternalInput")
    wq_out = nc.dram_tensor("wq", [BL, KD], BF16, kind="ExternalOutput")
    at_out = nc.dram_tensor("attn", [BL, D], BF16, kind="ExternalOutput")

    # Gram entry -> column in the per-group G tile [P, 10, GRP].
    # 0..3: G00,G11,G22,G33 ; 4:G10 5:G20 6:G30 7:G21 8:G31 9:G32
    GOFF = {(1, 0): 4, (2, 0): 5, (3, 0): 6, (2, 1): 7, (3, 1): 8, (3, 2): 9}

    with tile.TileContext(nc) as tc:
        with (
            tc.tile_pool(name="singles", bufs=1) as singles,
            tc.tile_pool(name="work", bufs=2) as work,
            tc.tile_pool(name="small", bufs=3) as small,
            tc.tile_pool(name="sgrp", bufs=GRP) as sgrp,
            tc.tile_pool(name="grp", bufs=2) as grp,
            tc.tile_pool(name="psum", bufs=2, space="PSUM") as psum,
        ):
            # ---------------- one-time loads ----------------
            xT_sb = singles.tile([P, DC, BL], BF16)
            nc.sync.dma_start(out=xT_sb, in_=xT.ap().rearrange("(c p) e -> p c e", p=P))
            w1_sb = singles.tile([P, DC, H1], BF16)
            nc.sync.dma_start(out=w1_sb, in_=w1.ap().rearrange("(c p) n -> p c n", p=P))
            w2_sb = singles.tile([P, H1C, H2], BF16)
            nc.sync.dma_start(out=w2_sb, in_=w2.ap().rearrange("(c p) n -> p c n", p=P))
            wd_sb = singles.tile([P, H2C, KD], BF16)
            nc.sync.dma_start(out=wd_sb, in_=wd.ap().rearrange("(c p) n -> p c n", p=P))
            wa_sb = singles.tile([P, H2C, D], BF16)
            nc.sync.dma_start(out=wa_sb, in_=wa.ap().rearrange("(c p) n -> p c n", p=P))

            b1_sb = singles.tile([1, H1], BF16)
            nc.sync.dma_start(out=b1_sb, in_=b1r.ap())
            b2_sb = singles.tile([1, H2], BF16)
            nc.sync.dma_start(out=b2_sb, in_=b2r.ap())
            bd_sb = singles.tile([1, KD], BF16)
            nc.sync.dma_start(out=bd_sb, in_=bdr.ap())
            ba_sb = singles.tile([1, D], BF16)
            nc.sync.dma_start(out=ba_sb, in_=bar.ap())

            g1_sb = singles.tile([P, H1C], F32)
            nc.sync.dma_start(out=g1_sb, in_=g1c.ap())
            be1_sb = singles.tile([P, H1C], F32)
            nc.sync.dma_start(out=be1_sb, in_=be1c.ap())
            g2_sb = singles.tile([P, H2C], F32)
            nc.sync.dma_start(out=g2_sb, in_=g2c.ap())
            be2_sb = singles.tile([P, H2C], F32)
            nc.sync.dma_start(out=be2_sb, in_=be2c.ap())

            ones_sb = singles.tile([1, P], BF16)
            nc.vector.memset(ones_sb, 1.0)
            ident = singles.tile([P, P], BF16)
            make_identity(nc, ident)
            epst = singles.tile([P, 1], F32)
            nc.vector.memset(epst, 1e-5)

            def layer_norm_block(ps, width, cchunks, g_sb, be_sb, hT, tag):
                """psum [P,width] -> LN -> transpose -> relu-affine -> hT."""
                stats = small.tile([P, 6], F32, tag=tag + "_st")
                nc.vector.bn_stats(out=stats, in_=ps)
                mv = small.tile([P, 2], F32, tag=tag + "_mv")
                nc.vector.bn_aggr(out=mv, in_=stats)
                sd = small.tile([P, 1], F32, tag=tag + "_sd")
                nc.scalar.activation(sd, mv[:, 1:2], AF.Sqrt, bias=epst)
                rs = small.tile([P, 1], F32, tag=tag + "_rs")
                nc.vector.reciprocal(rs, sd)
                tcen = work.tile([P, width], BF16, tag=tag)
                nc.vector.tensor_scalar(
                    out=tcen, in0=ps, scalar1=mv[:, 0:1], scalar2=rs,
                    op0=AL.subtract, op1=AL.mult,
                )
                for c in range(cchunks):
                    pst = psum.tile([P, P], BF16, tag="tr", bufs=1)
                    nc.tensor.transpose(pst, tcen[:, ts(c, P)], ident)
                    nc.scalar.activation(
                        hT[:, c, :], pst, AF.Relu,
                        scale=g_sb[:, c : c + 1], bias=be_sb[:, c : c + 1],
                    )

            n_groups = T // GRP
            pending = None  # deferred (s_tiles, Gg, Spol, g) for pipelining
            for g in range(n_groups + 1):
              if g < n_groups:
                s_tiles = []
                Gg = grp.tile([P, 10, GRP], F32)
                # =========== phase A: per-tile pipeline up to Gram ===========
                for ti in range(GRP):
                    t = g * GRP + ti
                    # ---- layer 1 ----
                    ps1 = psum.tile([P, H1], F32, tag="mm")
                    nc.tensor.matmul(ps1, ones_sb, b1_sb, start=True, stop=False)
                    for c in range(DC):
                        nc.tensor.matmul(
                            ps1, xT_sb[:, c, ts(t, P)], w1_sb[:, c, :],
                            start=False, stop=(c == DC - 1),
                        )
                    h1T = work.tile([P, H1C, P], BF16)
                    layer_norm_block(ps1, H1, H1C, g1_sb, be1_sb, h1T, "tc1")

                    # ---- layer 2 ----
                    ps2 = psum.tile([P, H2], F32, tag="mm")
                    nc.tensor.matmul(ps2, ones_sb, b2_sb, start=True, stop=False)
                    for c in range(H1C):
                        nc.tensor.matmul(
                            ps2, h1T[:, c, :], w2_sb[:, c, :],
                            start=False, stop=(c == H1C - 1),
                        )
                    h2T = work.tile([P, H2C, P], BF16)
                    layer_norm_block(ps2, H2, H2C, g2_sb, be2_sb, h2T, "tc2")

                    # ---- W_flat = h2 @ Wd + bd ----
                    wf = work.tile([P, KD], BF16)
                    for n in range(KD // 512):
                        ps3 = psum.tile([P, 512], F32, tag="wf", bufs=3)
                        nc.tensor.matmul(ps3, ones_sb, bd_sb[:, ts(n, 512)],
                                         start=True, stop=False)
                        for c in range(H2C):
                            nc.tensor.matmul(
                                ps3, h2T[:, c, :], wd_sb[:, c, ts(n, 512)],
                                start=False, stop=(c == H2C - 1),
                            )
                        if n % 3 == 2:  # 2 of 6 copies on DVE, 4 on ACT
                            nc.vector.tensor_scalar(
                                out=wf[:, ts(n, 512)], in0=ps3, scalar1=1.0,
                                scalar2=None, op0=AL.mult,
                            )
                        else:
                            nc.scalar.activation(wf[:, ts(n, 512)], ps3, AF.Copy)

                    # ---- attn head ----
                    ps4 = psum.tile([P, D], F32, tag="mm4", bufs=1)
                    for n, nw in ((0, 512), (1, 256)):
                        sl = bass.ds(n * 512, nw)
                        nc.tensor.matmul(ps4[:, sl], ones_sb, ba_sb[:, sl],
                                         start=True, stop=False)
                        for c in range(H2C):
                            nc.tensor.matmul(
                                ps4[:, sl], h2T[:, c, :], wa_sb[:, c, sl],
                                start=False, stop=(c == H2C - 1),
                            )
                    # exp without max-subtraction (pre-activations are O(1))
                    ea = work.tile([P, D], BF16)
                    sumexp = small.tile([P, 1], F32)
                    nc.scalar.activation(ea, ps4, AF.Exp, accum_out=sumexp)
                    rden = small.tile([P, 1], F32)
                    nc.vector.reciprocal(rden, sumexp)
                    at = work.tile([P, D], BF16)
                    nc.gpsimd.tensor_scalar_mul(at, ea, rden)
                    nc.gpsimd.dma_start(out=at_out.ap()[ts(t, P), :], in_=at)

                    # ---- s_i = W_flat_i * ea (unnormalized attn) ----
                    s = sgrp.tile([P, K, D], BF16, tag="s")
                    s_tiles.append(s)
                    for i in range(K):
                        eng = nc.vector if i < 2 else nc.gpsimd
                        eng.tensor_tensor(
                            out=s[:, i, :], in0=wf[:, ts(i, D)], in1=ea, op=AL.mult
                        )

                    # ---- Gram entries ----
                    scrA = work.tile([P, D], BF16)  # ACT scratch
                    scrV = work.tile([P, D], BF16)  # DVE scratch
                    for i in (0, 1):  # diagonals on ScalarE
                        nc.scalar.activation(
                            scrA, s[:, i, :], AF.Square,
                            accum_out=Gg[:, i, ti : ti + 1],
                        )
                    for i in (2, 3):  # diagonals on DVE
                        prod = work.tile([P, D], BF16, tag="prod")
                        nc.vector.tensor_tensor(
                            out=prod, in0=s[:, i, :], in1=s[:, i, :], op=AL.mult
                        )
                        nc.vector.tensor_scalar(
                            out=scrV, in0=prod, scalar1=1.0, scalar2=None,
                            op0=AL.mult, op1=AL.add,
                            accum_out=Gg[:, i, ti : ti + 1],
                        )
                    for npair, ((i, j), c) in enumerate(GOFF.items()):
                        prod = work.tile([P, D], BF16, tag="prod")
                        eng = nc.gpsimd if npair % 2 == 0 else nc.vector
                        eng.tensor_tensor(
                            out=prod, in0=s[:, i, :], in1=s[:, j, :], op=AL.mult
                        )
                        nc.vector.tensor_scalar(
                            out=scrV, in0=prod, scalar1=1.0, scalar2=None,
                            op0=AL.mult, op1=AL.add,
                            accum_out=Gg[:, c, ti : ti + 1],
                        )

              if pending is None:
                pending = (s_tiles, Gg, Spol)
                continue
              ps_tiles, Gg_c, Spol_c = pending
              if g < n_groups:
                pending = (s_tiles, Gg, Spol)
              s_tiles, Gg, Spol = ps_tiles, Gg_c, Spol_c
              if True:
                # =========== phase B: batched Cholesky over the group ========
                # All ops on [P, GRP] f32 slices. M = -L offdiag;
                # M cols 0:M10 1:M20 2:M30 3:M21 4:M31 5:M32 ; r_i = 1/L_ii.
                Mg = grp.tile([P, 6, GRP], F32)
                rg = grp.tile([P, 4, GRP], F32)
                rng_ = grp.tile([P, 3, GRP], F32)
                tg = grp.tile([P, 8, GRP], F32)

                def GE(e):
                    return Gg[:, e, :]

                def ME(e):
                    return Mg[:, e, :]

                def TG(e):
                    return tg[:, e, :]

                def chol_sqrt(i, src, need_rn=True):
                    lv = small.tile([P, GRP], F32, tag=f"chl{i}")
                    nc.scalar.activation(lv, src, AF.Sqrt)
                    nc.vector.reciprocal(rg[:, i, :], lv)
                    if need_rn:
                        nc.vector.tensor_scalar_mul(
                            rng_[:, i, :], rg[:, i, :], -1.0
                        )

                chol_sqrt(0, GE(0))
                for e in range(3):  # M10,M20,M30 = -G{i0}*r0
                    nc.vector.tensor_tensor(
                        out=ME(e), in0=GE(4 + e), in1=rng_[:, 0, :], op=AL.mult
                    )
                nc.vector.tensor_tensor(out=TG(0), in0=ME(0), in1=ME(0), op=AL.mult)
                nc.vector.tensor_tensor(out=TG(1), in0=GE(1), in1=TG(0), op=AL.subtract)
                chol_sqrt(1, TG(1))
                # L21num = G21 - M20*M10 ; M21 = -L21num*r1 (= L21num*rn1)
                nc.vector.tensor_tensor(out=TG(0), in0=ME(1), in1=ME(0), op=AL.mult)
                nc.vector.tensor_tensor(out=TG(2), in0=GE(7), in1=TG(0), op=AL.subtract)
                nc.vector.tensor_tensor(out=ME(3), in0=TG(2), in1=rng_[:, 1, :], op=AL.mult)
                # L31num = G31 - M30*M10 ; M31
                nc.vector.tensor_tensor(out=TG(0), in0=ME(2), in1=ME(0), op=AL.mult)
                nc.vector.tensor_tensor(out=TG(3), in0=GE(8), in1=TG(0), op=AL.subtract)
                nc.vector.tensor_tensor(out=ME(4), in0=TG(3), in1=rng_[:, 1, :], op=AL.mult)
                # d2 = G22 - M20^2 - M21^2
                nc.vector.tensor_tensor(out=TG(0), in0=ME(1), in1=ME(1), op=AL.mult)
                nc.vector.tensor_tensor(out=TG(4), in0=GE(2), in1=TG(0), op=AL.subtract)
                nc.vector.tensor_tensor(out=TG(0), in0=ME(3), in1=ME(3), op=AL.mult)
                nc.vector.tensor_tensor(out=TG(5), in0=TG(4), in1=TG(0), op=AL.subtract)
                chol_sqrt(2, TG(5))
                # L32num = G32 - M30*M20 - M31*M21 ; M32
                nc.vector.tensor_tensor(out=TG(0), in0=ME(2), in1=ME(1), op=AL.mult)
                nc.vector.tensor_tensor(out=TG(6), in0=GE(9), in1=TG(0), op=AL.subtract)
                nc.vector.tensor_tensor(out=TG(0), in0=ME(4), in1=ME(3), op=AL.mult)
                nc.vector.tensor_tensor(out=TG(7), in0=TG(6), in1=TG(0), op=AL.subtract)
                nc.vector.tensor_tensor(out=ME(5), in0=TG(7), in1=rng_[:, 2, :], op=AL.mult)
                # d3 = G33 - M30^2 - M31^2 - M32^2
                nc.vector.tensor_tensor(out=TG(0), in0=ME(2), in1=ME(2), op=AL.mult)
                nc.vector.tensor_tensor(out=TG(1), in0=GE(3), in1=TG(0), op=AL.subtract)
                nc.vector.tensor_tensor(out=TG(0), in0=ME(4), in1=ME(4), op=AL.mult)
                nc.vector.tensor_tensor(out=TG(2), in0=TG(1), in1=TG(0), op=AL.subtract)
                nc.vector.tensor_tensor(out=TG(0), in0=ME(5), in1=ME(5), op=AL.mult)
                nc.vector.tensor_tensor(out=TG(3), in0=TG(2), in1=TG(0), op=AL.subtract)
                chol_sqrt(3, TG(3), need_rn=False)

                # =========== phase C: reconstruction =========================
                for ti in range(GRP):
                    t = g * GRP + ti
                    s = s_tiles[ti]
                    qt = work.tile([P, KD], BF16)

                    def q(i):
                        return qt[:, ts(i, D)]

                    def M(e):
                        return Mg[:, e, ti : ti + 1]

                    def r(i):
                        return rg[:, i, ti : ti + 1]

                    nc.vector.tensor_scalar_mul(q(0), s[:, 0, :], r(0))
                    nc.vector.scalar_tensor_tensor(
                        out=q(1), in0=q(0), scalar=M(0), in1=s[:, 1, :],
                        op0=AL.mult, op1=AL.add,
                    )
                    nc.vector.tensor_scalar_mul(q(1), q(1), r(1))
                    nc.vector.scalar_tensor_tensor(
                        out=q(2), in0=q(0), scalar=M(1), in1=s[:, 2, :],
                        op0=AL.mult, op1=AL.add,
                    )
                    nc.vector.scalar_tensor_tensor(
                        out=q(2), in0=q(1), scalar=M(3), in1=q(2),
                        op0=AL.mult, op1=AL.add,
                    )
                    nc.vector.tensor_scalar_mul(q(2), q(2), r(2))
                    nc.vector.scalar_tensor_tensor(
                        out=q(3), in0=q(0), scalar=M(2), in1=s[:, 3, :],
                        op0=AL.mult, op1=AL.add,
                    )
                    nc.vector.scalar_tensor_tensor(
                        out=q(3), in0=q(1), scalar=M(4), in1=q(3),
                        op0=AL.mult, op1=AL.add,
                    )
                    nc.vector.scalar_tensor_tensor(
                        out=q(3), in0=q(2), scalar=M(5), in1=q(3),
                        op0=AL.mult, op1=AL.add,
                    )
                    nc.vector.tensor_scalar_mul(q(3), q(3), r(3))

                    nc.scalar.dma_start(out=wq_out.ap()[ts(t, P), :], in_=qt)

    return nc


_CACHE = {}


def _get_nc(BL):
    if BL not in _CACHE:
        nc = bacc.Bacc("TRN2", target_bir_lowering=False, debug=False,
                       num_devices=N_CORES)
        build(nc, BL)
        nc.compile()
        _CACHE[BL] = nc
    return _CACHE[BL]


def make_in_maps(embeddings, W1, b1, g1, be1, W2, b2, g2, be2, Wd, bd, Wa, ba,
                 n_cores=N_CORES):
    """Shard the batch, replicate (bf16-converted, re-laid-out) params."""
    embeddings = np.asarray(embeddings, np.float32)
    BL = embeddings.shape[0] // n_cores

    def col_layout(v, chunks):
        return np.ascontiguousarray(
            np.asarray(v, np.float32).reshape(chunks, P).T
        )

    common = {
        "w1": np.ascontiguousarray(np.asarray(W1).astype(NPBF16)),
        "w2": np.ascontiguousarray(np.asarray(W2).astype(NPBF16)),
        "wd": np.ascontiguousarray(np.asarray(Wd).astype(NPBF16)),
        "wa": np.ascontiguousarray(np.asarray(Wa).astype(NPBF16)),
        "b1r": np.asarray(b1).astype(NPBF16).reshape(1, H1),
        "b2r": np.asarray(b2).astype(NPBF16).reshape(1, H2),
        "bdr": np.asarray(bd).astype(NPBF16).reshape(1, KD),
        "bar": np.asarray(ba).astype(NPBF16).reshape(1, D),
        "g1c": col_layout(g1, H1 // P),
        "be1c": col_layout(be1, H1 // P),
        "g2c": col_layout(g2, H2 // P),
        "be2c": col_layout(be2, H2 // P),
    }
    in_maps = []
    for i in range(n_cores):
        shard = embeddings[i * BL : (i + 1) * BL]
        xTs = np.ascontiguousarray(shard.astype(NPBF16).T)
        in_maps.append({"xT": xTs, **common})
    return in_maps, BL


def kernel(embeddings, W1, b1, g1, be1, W2, b2, g2, be2, Wd, bd, Wa, ba):
    in_maps, BL = make_in_maps(
        embeddings, W1, b1, g1, be1, W2, b2, g2, be2, Wd, bd, Wa, ba
    )
    nc = _get_nc(BL)
    res = run_bass_kernel_spmd(nc, in_maps, core_ids=list(range(N_CORES)))
    wq = np.concatenate(
        [np.asarray(res.results[i]["wq"]).astype(np.float32) for i in range(N_CORES)],
        axis=0,
    ).reshape(-1, K, D)
    at = np.concatenate(
        [np.asarray(res.results[i]["attn"]).astype(np.float32) for i in range(N_CORES)],
        axis=0,
    )
    return wq, at
